# revision 28
# baseline (speedup 1.0000x reference)
"""LocationAttention Trainium2 kernel (nn_LocationAttention_83485574300223).

out[b,t,:] = sum_{s<=t} a[b,s] x[b,s,:] / (sum_{s<=t} a[b,s] + eps),
a = exp(x @ w + b).

Data-parallel over batch: 16 -> 2 per core, 8 cores. fp16 I/O with a
host-permuted layout [128, 33, 512]: partition p holds token c*127+p-1 of
chunk c (partition 0 = carry slot, host-zeroed; tail zero-padded), giving
8KB-contiguous DMA descriptors per partition. Causal prefix per 127-token
chunk: lhsT = triu mask (shifted) with row 0 = carry-in and col 0 =
carry-out, scaled per-partition by a; psum row 0 is the next carry and
copies partition-0-aligned into the next chunk's rhs row 0. Two-chunk
superblocks: carry enters at even chunks only; odd chunks add the even
chunk's contribution via a rank-structured pair matmul (columns = [1; a]).
Den rides the same weights via N=1 matmuls on ones-columns. DMA-bound.
"""
import numpy as np

import concourse.bass as bass
import concourse.tile as tile
from concourse import mybir
from concourse.bass_utils import run_bass_kernel_spmd

B, S, H = 16, 4096, 512
NCORES = 8
BPC = B // NCORES   # batch elements per core
CL = 127            # tokens per chunk (partition 0 = carry slot)
NCH = 33            # chunks per batch element (33*127 = 4191 >= 4096, padded)
SPAD = NCH * CL     # padded token count (4191)
GSTART = (0, 8, 16, 24)
GSIZE = (8, 8, 8, 9)
NG = 4              # DMA groups per batch element

F32 = mybir.dt.float32
F16 = mybir.dt.float16
AF = mybir.ActivationFunctionType
ALU = mybir.AluOpType


def _split_multiwaits(nc, limit=1):
    """This walrus build accepts at most one sync-wait per instruction.
    Split extras into preceding single-wait NoOps on the same engine."""
    for fn in nc.m.functions:
        for bb in fn.blocks:
            out = []
            changed = False
            for ins in bb.instructions:
                si = getattr(ins, "sync_info", None)
                waits = list(si.on_wait) if (si is not None and si.on_wait) else []
                if len(waits) > limit:
                    extra, keep = waits[:-limit], waits[-limit:]
                    for i, w in enumerate(extra):
                        nop = mybir.InstNoOp(name=f"{ins.name}-ws{i}", ins=[], outs=[])
                        nop.engine = ins.engine
                        nop.sync_info = mybir.SyncInfo(on_wait=[w], on_update=[])
                        out.append(nop)
                    si.on_wait = keep
                    changed = True
                out.append(ins)
            if changed:
                try:
                    bb.instructions = out
                except Exception:
                    bb.instructions.clear()
                    bb.instructions.extend(out)


def _chunk_group(c):
    g = min(c // 8, NG - 1)
    return g, c - GSTART[g]


def _build(reps=1, split_multiwaits=True):
    nc = bass.Bass()
    x = nc.declare_dram_parameter("x", [BPC, 128, NCH, H], F16, isOutput=False)
    triD = nc.declare_dram_parameter("triD", [128, 128], F16, isOutput=False)
    wbv = nc.declare_dram_parameter("wbv", [1, H], F16, isOutput=False)
    bsc = nc.declare_dram_parameter("bsc", [1, 1], F32, isOutput=False)
    out = nc.declare_dram_parameter("out", [BPC, 128, NCH, H], F16, isOutput=True)

    with tile.TileContext(nc) as tc:
        with (
            tc.tile_pool(name="singles", bufs=1) as singles,
            tc.tile_pool(name="xp", bufs=6) as xp,
            tc.tile_pool(name="ogp", bufs=5) as ogp,
            tc.tile_pool(name="xwp", bufs=2) as xwp,
            tc.tile_pool(name="triap", bufs=6) as triap,
            tc.tile_pool(name="smallp", bufs=6) as smallp,
            tc.tile_pool(name="onesp", bufs=1) as onesp,
            tc.tile_pool(name="rp", bufs=1) as rp,
            tc.tile_pool(name="npp", bufs=3, space="PSUM") as npp,
            tc.tile_pool(name="dpp", bufs=1, space="PSUM") as dpp,
        ):
            # ---- constants ----
            triD_t = singles.tile([128, 128], F16)
            nc.sync.dma_start(out=triD_t, in_=triD[:])
            wb = singles.tile([128, H], F16)
            nc.sync.dma_start(out=wb, in_=wbv[:].to_broadcast([128, H]))
            b_sb = singles.tile([128, 1], F32)
            nc.sync.dma_start(out=b_sb, in_=bsc[:].to_broadcast([128, 1]))
            ones128 = singles.tile([128, 128], F16)
            nc.vector.memset(ones128[:, :], 1.0)
            ones_z = singles.tile([128, 1], F16)  # den column, zero carry slot
            nc.vector.memset(ones_z[:, :], 1.0)
            nc.vector.memset(ones_z[0:1, :], 0.0)

            for rep in range(reps):
                ones_c = {}
                den_ps = {}
                r4 = {}
                xts = {}    # (b, g) -> group tile [128, gsz, H]
                ogs = {}    # (b, g) -> out tile
                trias = {}
                pairs = {}
                a4s = {}
                psn = {}

                def _load(b, g, slices=1, eng=None):
                    gsz = GSIZE[g]
                    t = xp.tile([128, gsz, H], F16, tag="xt",
                                name=f"xt{rep}_{b}_{g}", padded_shape=[128, 9, H])
                    eng = eng or nc.sync
                    cuts = [gsz * i // slices for i in range(slices + 1)]
                    for lo, hi in zip(cuts[:-1], cuts[1:]):
                        eng.dma_start(
                            out=t[:, lo:hi, :],
                            in_=x[b][:, GSTART[g] + lo:GSTART[g] + hi, :])
                    xts[(b, g)] = t

                for b in range(BPC):
                    oc = onesp.tile([128, 1], F16, tag=f"on{b}", name=f"on{rep}_{b}")
                    nc.vector.memset(oc[:, :], 1.0)
                    nc.vector.memset(oc[0:1, :], 0.0)
                    ones_c[b] = oc
                    den_ps[b] = dpp.tile([128, NCH], F32, tag=f"d{b}",
                                         name=f"dp{rep}_{b}")
                    r4[b] = rp.tile([128, NCH], F32, tag=f"r{b}", name=f"r{rep}_{b}")
                # parallel fill: batch 0 on SP, batch 1 on Pool, 2-chunk slices
                _load(0, 0, slices=4, eng=nc.sync)
                _load(1, 0, slices=4, eng=nc.gpsimd)
                _load(0, 1, slices=2, eng=nc.sync)
                _load(1, 1, slices=2, eng=nc.gpsimd)

                LA = 2  # A-phase lookahead (chunks)

                def _rhs(b, c):
                    g, f = _chunk_group(c)
                    return xts[(b, g)][:, f, :]

                def _phase_a(b, c):
                    """score production for chunk c: STT dot, exp, lhsT build."""
                    xt_c = _rhs(b, c)
                    xw = xwp.tile([128, H], F16, tag="xw", name=f"xw{rep}_{b}_{c}")
                    p4 = smallp.tile([128, 1], F32, tag="p4", name=f"p4{rep}_{b}_{c}")
                    stt_eng = nc.gpsimd if c % 4 == 3 else nc.vector
                    stt_eng.scalar_tensor_tensor(
                        out=xw[:, :], in0=xt_c[:, :], scalar=1.0,
                        in1=wb[:, :], op0=ALU.mult, op1=ALU.mult,
                        accum_out=p4[:, 0:1],
                    )
                    a4 = smallp.tile([128, 1], F32, tag="a4", name=f"a4{rep}_{b}_{c}")
                    nc.scalar.activation(
                        out=a4[:, :], in_=p4[:, :], func=AF.Exp,
                        bias=b_sb[:, 0:1],
                    )
                    nc.gpsimd.memset(a4[0:1, :], 1.0)
                    tria = triap.tile([128, 128], F16, tag="tria",
                                      name=f"tria{rep}_{b}_{c}")
                    nc.gpsimd.tensor_scalar_mul(tria[:, :], triD_t[:, :], a4[:, 0:1])
                    if c % 2 == 1:  # pair lhsT: every column = [1; a_{c-1}]
                        pairT = triap.tile([128, 128], F16, tag="pairT",
                                           name=f"pairT{rep}_{b}_{c}", bufs=4)
                        a_prev = a4s.pop((b, c - 1))
                        nc.gpsimd.tensor_scalar_mul(
                            pairT[:, :], ones128[:, :], a_prev[:, 0:1])
                        pairs[(b, c)] = pairT
                    a4s[(b, c)] = a4
                    return tria

                def _scale(b, c, on_dve=False):
                    g, f = _chunk_group(c)
                    ps = psn.pop((b, c))
                    og = ogs.get((b, g))
                    if og is None:
                        og = ogp.tile([128, GSIZE[g], H], F16, tag="og",
                                      name=f"og{rep}_{b}_{g}",
                                      padded_shape=[128, 9, H])
                        ogs[(b, g)] = og
                    if on_dve:
                        nc.vector.tensor_scalar_mul(
                            og[:, f, :], ps[:, :], r4[b][:, c:c + 1])
                    else:
                        nc.scalar.activation(
                            out=og[:, f, :], in_=ps[:, :], func=AF.Copy,
                            scale=r4[b][:, c:c + 1],
                        )
                    # store in slices as scales complete (finer near the end to
                    # shorten the drain)
                    step = 2 if g == NG - 1 else 3
                    if f % step == step - 1 or f == GSIZE[g] - 1:
                        lo = (f // step) * step
                        nc.sync.dma_start(
                            out=out[b][:, GSTART[g] + lo:GSTART[g] + f + 1, :],
                            in_=og[:, lo:f + 1, :])

                def _phase_b(b, c):
                    """chain for chunk c: matmuls, carry hand-off, recip, scale."""
                    tria = trias.pop((b, c))
                    xt_c = _rhs(b, c)
                    odd = c % 2 == 1
                    ps = npp.tile([128, H], F32, tag=f"np{b}", name=f"ps{rep}_{b}_{c}")
                    psn[(b, c)] = ps
                    # den column; even chunks read the carry slot, odd chunks add
                    # the even chunk's total via the pair lhsT
                    nc.tensor.matmul(
                        den_ps[b][:, c:c + 1], tria[:, :],
                        (ones_z if odd else ones_c[b])[:, 0:1],
                        start=True, stop=not odd,
                    )
                    nc.tensor.matmul(
                        ps[:, :], tria[:, :], xt_c[:, :],
                        start=True, stop=not odd,
                    )
                    if odd:
                        pairT = pairs.pop((b, c))
                        nc.tensor.matmul(
                            den_ps[b][:, c:c + 1], pairT[:, :], ones_c[b][:, 0:1],
                            start=False, stop=True,
                        )
                        nc.tensor.matmul(
                            ps[:, :], pairT[:, :], _rhs(b, c - 1)[:, :],
                            start=False, stop=True,
                        )
                        if c + 1 < NCH:
                            nc.vector.tensor_copy(
                                ones_c[b][0:1, 0:1], den_ps[b][0:1, c:c + 1])
                        nc.vector.reciprocal(
                            r4[b][:, c - 1:c + 1], den_ps[b][:, c - 1:c + 1])
                        if c + 1 < NCH:
                            # r=1 on the carry row: the scale below then lands the
                            # raw carry (psum row 0) in og row 0, from where a
                            # legal SBUF->SBUF copy hands it to the next chunk
                            nc.vector.memset(r4[b][0:1, c:c + 1], 1.0)
                        _scale(b, c - 1, on_dve=(c // 2) % 3 == 2)
                        _scale(b, c)
                        if c + 1 < NCH:
                            g, f = _chunk_group(c)
                            nc.gpsimd.tensor_copy(
                                _rhs(b, c + 1)[0:1, 0:H], ogs[(b, g)][0:1, f, :])
                    elif c == NCH - 1:  # lone final even chunk
                        nc.vector.reciprocal(
                            r4[b][:, c:c + 1], den_ps[b][:, c:c + 1])
                        _scale(b, c)

                # prologue: A for chunks 0..LA-1
                for c in range(LA):
                    for b in range(BPC):
                        trias[(b, c)] = _phase_a(b, c)
                # steady state
                for c in range(NCH):
                    for b in range(BPC):
                        if c % 8 == 0 and c // 8 + 2 < NG:
                            g = c // 8 + 2
                            if g == NG - 1:  # last group off SP, sliced
                                _load(b, g, slices=4, eng=nc.gpsimd)
                            else:
                                _load(b, g)
                        if c + LA < NCH:
                            trias[(b, c + LA)] = _phase_a(b, c + LA)
                        _phase_b(b, c)

    if split_multiwaits:
        _split_multiwaits(nc)
    return nc


_NC = {}


def _get_nc(reps=1):
    if reps not in _NC:
        _NC[reps] = _build(reps)
    return _NC[reps]


def _prep_in_maps(input_data, w, b):
    xin = np.asarray(input_data)
    assert xin.shape == (B, S, H), xin.shape
    x16 = xin.astype(np.float16)
    # permuted layout: xr[b, p, c, :] = token c*127 + p - 1 (p>=1), zero-padded
    xpad = np.zeros((B, SPAD, H), np.float16)
    xpad[:, :S] = x16
    xr = np.zeros((B, 128, NCH, H), np.float16)
    xr[:, 1:128] = xpad.reshape(B, NCH, CL, H).transpose(0, 2, 1, 3)
    w16 = np.asarray(w, dtype=np.float16).reshape(1, H)
    bsc = np.asarray(b, dtype=np.float32).reshape(1, 1)
    # [carry; tokens] causal mask: row 0 = carry-in (ones), col 0 = carry-out
    # (ones), interior = shifted upper-triangular token mask.
    triD = np.zeros((128, 128), np.float32)
    triD[0, :] = 1.0
    triD[:, 0] = 1.0
    triD[1:, 1:] = np.triu(np.ones((CL, CL), np.float32))
    triD = triD.astype(np.float16)
    return [
        {
            "x": np.ascontiguousarray(xr[i * BPC:(i + 1) * BPC]),
            "triD": triD,
            "wbv": w16,
            "bsc": bsc,
        }
        for i in range(NCORES)
    ]


def _gather(results):
    outs = np.concatenate(
        [results[i]["out"] for i in range(NCORES)], axis=0)  # [B,128,NCH,H] f16
    # un-permute: out[b, c*127+p-1] = outs[b, p, c]
    opad = outs[:, 1:128].transpose(0, 2, 1, 3).reshape(B, SPAD, H)
    return opad[:, :S].astype(np.float32)


def _run(input_data, w, b, trace=False, reps=1):
    nc = _get_nc(reps)
    in_maps = _prep_in_maps(input_data, w, b)
    res = run_bass_kernel_spmd(
        nc, in_maps, core_ids=list(range(NCORES)), trace=trace
    )
    return _gather(res.results), res


def kernel(input_data, w, b):
    out, _ = _run(input_data, w, b, trace=False)
    return out


# revision 32
# speedup vs baseline: 1564.1515x; 1564.1515x over previous
"""LocationAttention Trainium2 kernel (nn_LocationAttention_83485574300223).

out[b,t,:] = sum_{s<=t} a[b,s] x[b,s,:] / (sum_{s<=t} a[b,s] + eps),
a = exp(x @ w + b).

Data-parallel over batch: 16 -> 2 per core, 8 cores. fp16 I/O with a
host-permuted layout [128, 33, 512]: partition p holds token c*127+p-1 of
chunk c (partition 0 = carry slot, host-zeroed; tail zero-padded), giving
8KB-contiguous DMA descriptors per partition. Causal prefix per 127-token
chunk: lhsT = triu mask (shifted) with row 0 = carry-in and col 0 =
carry-out, scaled per-partition by a; psum row 0 is the next carry and
copies partition-0-aligned into the next chunk's rhs row 0. Two-chunk
superblocks: carry enters at even chunks only; odd chunks add the even
chunk's contribution via a rank-structured pair matmul (columns = [1; a]).
Den rides the same weights via N=1 matmuls on ones-columns. DMA-bound.
"""
import numpy as np

import concourse.bass as bass
import concourse.tile as tile
from concourse import mybir
from concourse.bass_utils import run_bass_kernel_spmd

B, S, H = 16, 4096, 512
NCORES = 8
BPC = B // NCORES   # batch elements per core
CL = 127            # tokens per chunk (partition 0 = carry slot)
NCH = 33            # chunks per batch element (33*127 = 4191 >= 4096, padded)
SPAD = NCH * CL     # padded token count (4191)
GSTART = (0, 8, 16, 24)
GSIZE = (8, 8, 8, 9)
NG = 4              # DMA groups per batch element

F32 = mybir.dt.float32
F16 = mybir.dt.float16
AF = mybir.ActivationFunctionType
ALU = mybir.AluOpType

GPS_COMPUTE = True  # False: route all gpsimd compute ops to DVE (diagnostic)


def _split_multiwaits(nc, limit=1):
    """This walrus build accepts at most one sync-wait per instruction.
    Split extras into preceding single-wait NoOps on the same engine."""
    for fn in nc.m.functions:
        for bb in fn.blocks:
            out = []
            changed = False
            for ins in bb.instructions:
                si = getattr(ins, "sync_info", None)
                waits = list(si.on_wait) if (si is not None and si.on_wait) else []
                if len(waits) > limit:
                    extra, keep = waits[:-limit], waits[-limit:]
                    for i, w in enumerate(extra):
                        nop = mybir.InstNoOp(name=f"{ins.name}-ws{i}", ins=[], outs=[])
                        nop.engine = ins.engine
                        nop.sync_info = mybir.SyncInfo(on_wait=[w], on_update=[])
                        out.append(nop)
                    si.on_wait = keep
                    changed = True
                out.append(ins)
            if changed:
                try:
                    bb.instructions = out
                except Exception:
                    bb.instructions.clear()
                    bb.instructions.extend(out)


def _chunk_group(c):
    g = min(c // 8, NG - 1)
    return g, c - GSTART[g]


def _build(reps=1, split_multiwaits=True):
    nc = bass.Bass()
    x = nc.declare_dram_parameter("x", [BPC, 128, NCH, H], F16, isOutput=False)
    triD = nc.declare_dram_parameter("triD", [128, 128], F16, isOutput=False)
    wbv = nc.declare_dram_parameter("wbv", [1, H], F16, isOutput=False)
    bsc = nc.declare_dram_parameter("bsc", [1, 1], F32, isOutput=False)
    out = nc.declare_dram_parameter("out", [BPC, 128, NCH, H], F16, isOutput=True)

    with tile.TileContext(nc) as tc:
        with (
            tc.tile_pool(name="singles", bufs=1) as singles,
            tc.tile_pool(name="xp", bufs=6) as xp,
            tc.tile_pool(name="ogp", bufs=5) as ogp,
            tc.tile_pool(name="xwp", bufs=2) as xwp,
            tc.tile_pool(name="triap", bufs=6) as triap,
            tc.tile_pool(name="smallp", bufs=6) as smallp,
            tc.tile_pool(name="onesp", bufs=1) as onesp,
            tc.tile_pool(name="rp", bufs=1) as rp,
            tc.tile_pool(name="npp", bufs=3, space="PSUM") as npp,
            tc.tile_pool(name="dpp", bufs=1, space="PSUM") as dpp,
        ):
            # ---- constants ----
            triD_t = singles.tile([128, 128], F16)
            nc.sync.dma_start(out=triD_t, in_=triD[:])
            wb = singles.tile([128, H], F16)
            nc.sync.dma_start(out=wb, in_=wbv[:].to_broadcast([128, H]))
            b_sb = singles.tile([128, 1], F32)
            nc.sync.dma_start(out=b_sb, in_=bsc[:].to_broadcast([128, 1]))
            ones128 = singles.tile([128, 128], F16)
            nc.vector.memset(ones128[:, :], 1.0)
            ones_z = singles.tile([128, 1], F16)  # den column, zero carry slot
            nc.vector.memset(ones_z[:, :], 1.0)
            nc.vector.memset(ones_z[0:1, :], 0.0)

            for rep in range(reps):
                ones_c = {}
                den_ps = {}
                r4 = {}
                xts = {}    # (b, g) -> group tile [128, gsz, H]
                ogs = {}    # (b, g) -> out tile
                trias = {}
                pairs = {}
                a4s = {}
                psn = {}

                def _load(b, g, slices=1, eng=None):
                    gsz = GSIZE[g]
                    t = xp.tile([128, gsz, H], F16, tag="xt",
                                name=f"xt{rep}_{b}_{g}", padded_shape=[128, 9, H])
                    eng = eng or nc.sync
                    cuts = [gsz * i // slices for i in range(slices + 1)]
                    for lo, hi in zip(cuts[:-1], cuts[1:]):
                        eng.dma_start(
                            out=t[:, lo:hi, :],
                            in_=x[b][:, GSTART[g] + lo:GSTART[g] + hi, :])
                    xts[(b, g)] = t

                for b in range(BPC):
                    oc = onesp.tile([128, 1], F16, tag=f"on{b}", name=f"on{rep}_{b}")
                    nc.vector.memset(oc[:, :], 1.0)
                    nc.vector.memset(oc[0:1, :], 0.0)
                    ones_c[b] = oc
                    den_ps[b] = dpp.tile([128, NCH], F32, tag=f"d{b}",
                                         name=f"dp{rep}_{b}")
                    r4[b] = rp.tile([128, NCH], F32, tag=f"r{b}", name=f"r{rep}_{b}")
                # parallel fill: batch 0 on SP, batch 1 on Pool, 2-chunk slices
                _load(0, 0, slices=4, eng=nc.sync)
                _load(1, 0, slices=4, eng=nc.gpsimd)
                _load(0, 1, slices=2, eng=nc.sync)
                _load(1, 1, slices=2, eng=nc.gpsimd)

                LA = 2  # A-phase lookahead (chunks)

                def _rhs(b, c):
                    g, f = _chunk_group(c)
                    return xts[(b, g)][:, f, :]

                def _phase_a(b, c):
                    """score production for chunk c: STT dot, exp, lhsT build."""
                    xt_c = _rhs(b, c)
                    gps = nc.gpsimd if GPS_COMPUTE else nc.vector
                    xw = xwp.tile([128, H], F16, tag="xw", name=f"xw{rep}_{b}_{c}")
                    p4 = smallp.tile([128, 1], F32, tag="p4", name=f"p4{rep}_{b}_{c}")
                    nc.vector.scalar_tensor_tensor(
                        out=xw[:, :], in0=xt_c[:, :], scalar=1.0,
                        in1=wb[:, :], op0=ALU.mult, op1=ALU.mult,
                        accum_out=p4[:, 0:1],
                    )
                    # keep exp(junk carry-row dot) finite: the row-0 logit is
                    # never used, but an Inf would raise HW error notifications
                    nc.vector.memset(p4[0:1, :], 0.0)
                    a4 = smallp.tile([128, 1], F32, tag="a4", name=f"a4{rep}_{b}_{c}")
                    nc.scalar.activation(
                        out=a4[:, :], in_=p4[:, :], func=AF.Exp,
                        bias=b_sb[:, 0:1],
                    )
                    gps.memset(a4[0:1, :], 1.0)
                    tria = triap.tile([128, 128], F16, tag="tria",
                                      name=f"tria{rep}_{b}_{c}")
                    gps.tensor_scalar_mul(tria[:, :], triD_t[:, :], a4[:, 0:1])
                    if c % 2 == 1:  # pair lhsT: every column = [1; a_{c-1}]
                        pairT = triap.tile([128, 128], F16, tag="pairT",
                                           name=f"pairT{rep}_{b}_{c}", bufs=4)
                        a_prev = a4s.pop((b, c - 1))
                        gps.tensor_scalar_mul(
                            pairT[:, :], ones128[:, :], a_prev[:, 0:1])
                        pairs[(b, c)] = pairT
                    a4s[(b, c)] = a4
                    return tria

                def _scale(b, c, on_dve=False):
                    g, f = _chunk_group(c)
                    ps = psn.pop((b, c))
                    og = ogs.get((b, g))
                    if og is None:
                        og = ogp.tile([128, GSIZE[g], H], F16, tag="og",
                                      name=f"og{rep}_{b}_{g}",
                                      padded_shape=[128, 9, H])
                        ogs[(b, g)] = og
                    if on_dve:
                        nc.vector.tensor_scalar_mul(
                            og[:, f, :], ps[:, :], r4[b][:, c:c + 1])
                    else:
                        nc.scalar.activation(
                            out=og[:, f, :], in_=ps[:, :], func=AF.Copy,
                            scale=r4[b][:, c:c + 1],
                        )
                    # store in slices as scales complete (finer near the end to
                    # shorten the drain)
                    step = 2 if g == NG - 1 else 3
                    if f % step == step - 1 or f == GSIZE[g] - 1:
                        lo = (f // step) * step
                        nc.sync.dma_start(
                            out=out[b][:, GSTART[g] + lo:GSTART[g] + f + 1, :],
                            in_=og[:, lo:f + 1, :])

                def _phase_b(b, c):
                    """chain for chunk c: matmuls, carry hand-off, recip, scale."""
                    tria = trias.pop((b, c))
                    xt_c = _rhs(b, c)
                    odd = c % 2 == 1
                    ps = npp.tile([128, H], F32, tag=f"np{b}", name=f"ps{rep}_{b}_{c}")
                    psn[(b, c)] = ps
                    # den column; even chunks read the carry slot, odd chunks add
                    # the even chunk's total via the pair lhsT
                    nc.tensor.matmul(
                        den_ps[b][:, c:c + 1], tria[:, :],
                        (ones_z if odd else ones_c[b])[:, 0:1],
                        start=True, stop=not odd,
                    )
                    nc.tensor.matmul(
                        ps[:, :], tria[:, :], xt_c[:, :],
                        start=True, stop=not odd,
                    )
                    if odd:
                        pairT = pairs.pop((b, c))
                        nc.tensor.matmul(
                            den_ps[b][:, c:c + 1], pairT[:, :], ones_c[b][:, 0:1],
                            start=False, stop=True,
                        )
                        nc.tensor.matmul(
                            ps[:, :], pairT[:, :], _rhs(b, c - 1)[:, :],
                            start=False, stop=True,
                        )
                        if c + 1 < NCH:
                            nc.vector.tensor_copy(
                                ones_c[b][0:1, 0:1], den_ps[b][0:1, c:c + 1])
                        nc.vector.reciprocal(
                            r4[b][:, c - 1:c + 1], den_ps[b][:, c - 1:c + 1])
                        if c + 1 < NCH:
                            # r=1 on the carry row: the scale below then lands the
                            # raw carry (psum row 0) in og row 0, from where a
                            # legal SBUF->SBUF copy hands it to the next chunk
                            nc.vector.memset(r4[b][0:1, c:c + 1], 1.0)
                        _scale(b, c - 1, on_dve=(c // 2) % 3 == 2)
                        _scale(b, c)
                        if c + 1 < NCH:
                            g, f = _chunk_group(c)
                            nc.gpsimd.tensor_copy(
                                _rhs(b, c + 1)[0:1, 0:H], ogs[(b, g)][0:1, f, :])
                    elif c == NCH - 1:  # lone final even chunk
                        nc.vector.reciprocal(
                            r4[b][:, c:c + 1], den_ps[b][:, c:c + 1])
                        _scale(b, c)

                # prologue: A for chunks 0..LA-1
                for c in range(LA):
                    for b in range(BPC):
                        trias[(b, c)] = _phase_a(b, c)
                # steady state
                for c in range(NCH):
                    for b in range(BPC):
                        if c % 8 == 0 and c // 8 + 2 < NG:
                            _load(b, c // 8 + 2)
                        if c + LA < NCH:
                            trias[(b, c + LA)] = _phase_a(b, c + LA)
                        _phase_b(b, c)

    if split_multiwaits:
        _split_multiwaits(nc)
    return nc


_NC = {}


def _get_nc(reps=1):
    if reps not in _NC:
        _NC[reps] = _build(reps)
    return _NC[reps]


def _prep_in_maps(input_data, w, b):
    xin = np.asarray(input_data)
    assert xin.shape == (B, S, H), xin.shape
    x16 = xin.astype(np.float16)
    # permuted layout: xr[b, p, c, :] = token c*127 + p - 1 (p>=1), zero-padded
    xpad = np.zeros((B, SPAD, H), np.float16)
    xpad[:, :S] = x16
    xr = np.zeros((B, 128, NCH, H), np.float16)
    xr[:, 1:128] = xpad.reshape(B, NCH, CL, H).transpose(0, 2, 1, 3)
    w16 = np.asarray(w, dtype=np.float16).reshape(1, H)
    bsc = np.asarray(b, dtype=np.float32).reshape(1, 1)
    # [carry; tokens] causal mask: row 0 = carry-in (ones), col 0 = carry-out
    # (ones), interior = shifted upper-triangular token mask.
    triD = np.zeros((128, 128), np.float32)
    triD[0, :] = 1.0
    triD[:, 0] = 1.0
    triD[1:, 1:] = np.triu(np.ones((CL, CL), np.float32))
    triD = triD.astype(np.float16)
    return [
        {
            "x": np.ascontiguousarray(xr[i * BPC:(i + 1) * BPC]),
            "triD": triD,
            "wbv": w16,
            "bsc": bsc,
        }
        for i in range(NCORES)
    ]


def _gather(results):
    outs = np.concatenate(
        [results[i]["out"] for i in range(NCORES)], axis=0)  # [B,128,NCH,H] f16
    # un-permute: out[b, c*127+p-1] = outs[b, p, c]
    opad = outs[:, 1:128].transpose(0, 2, 1, 3).reshape(B, SPAD, H)
    return opad[:, :S].astype(np.float32)


def _run(input_data, w, b, trace=False, reps=1):
    nc = _get_nc(reps)
    in_maps = _prep_in_maps(input_data, w, b)
    res = run_bass_kernel_spmd(
        nc, in_maps, core_ids=list(range(NCORES)), trace=trace
    )
    return _gather(res.results), res


def kernel(input_data, w, b):
    out, _ = _run(input_data, w, b, trace=False)
    return out
